# revision 1
# baseline (speedup 1.0000x reference)
"""Trainium2 Bass kernel for BasicQuadRGBModel (quad-Bayer demosaic CNN).

v2 layout (engine APs need partition base in {0,32,64,96}; DMA is exempt):
  - im2col buffers R [120p, 10 rows, 64 win]: main block xa=1..8 at partitions
    (xa-1)*12+ci = [0:96); xa=0 strip at [96:108); xa=9 strip at [108:120).
    PSUM eviction is then a base0->base0 relu copy; strips are SBUF->SBUF DMAs.
  - grb/d buffers [20p]: main (xa-1)*2+c at [0:16); strips [16:18),[18:20).
  - layer-0 im2col r0 [128p] host-built: ky0 block [0:40), ky1-other [40:60),
    ky1-rb [64:84) (aligned: feeds d_buf copies), ky2 block [84:124).
  - conv = 3 accumulating matmuls/layer (K=120, M=96, N=512 = 8 rows x 64 win);
    softmax/green/chroma folded into small matmuls; float32r for full PE rate.
  - host does layer-0 im2col and the final 2x2 pixel-shuffle.
"""

import sys

sys.path.insert(0, "/opt/trn_rl_repo")

import numpy as np

import concourse.bass as bass
import concourse.mybir as mybir
import concourse.tile as tile
from concourse import bacc
from concourse.bass_utils import run_bass_kernel_spmd

N_CORES = 8
B_PC = 2
H = W = 512
NW = 64
NSLAB = 64
CH = 12
F32 = mybir.dt.float32
F32R = mybir.dt.float32r
USE_F32R = False


def _rbloc(xa, c):
    if xa == 0:
        return 16 + c
    if xa == 9:
        return 18 + c
    return (xa - 1) * 2 + c


def _rloc(xa, ci):
    if xa == 0:
        return 96 + ci
    if xa == 9:
        return 108 + ci
    return (xa - 1) * 12 + ci


def _r0loc(ky, ci, xa):
    if ky == 0:
        if ci == 0:
            return xa
        if ci == 3:
            return 10 + xa
        return 20 + _rbloc(xa, ci - 1)
    if ky == 1:
        if ci == 0:
            return 40 + xa
        if ci == 3:
            return 50 + xa
        return 64 + _rbloc(xa, ci - 1)
    if ci == 0:
        return 84 + xa
    if ci == 3:
        return 94 + xa
    return 104 + _rbloc(xa, ci - 1)


def build_r0(mosaic):
    B = mosaic.shape[0]
    mp = np.zeros((B, 4, H + 2, W + 2), np.float32)
    mp[:, :, 1 : H + 1, 1 : W + 1] = mosaic
    r0 = np.zeros((B, 128, H, NW), np.float32)
    for ky in range(3):
        for ci in range(4):
            for xa in range(10):
                r0[:, _r0loc(ky, ci, xa)] = mp[:, ci, ky : ky + H, xa : xa + 8 * NW : 8]
    return r0


def build_w_l0(wt):
    W_ = np.zeros((128, 96), np.float32)
    for ky in range(3):
        for ci in range(4):
            for xa in range(10):
                for xo in range(8):
                    kx = xa - xo
                    if 0 <= kx <= 2:
                        for co in range(CH):
                            W_[_r0loc(ky, ci, xa), xo * 12 + co] = wt[co, ci, ky, kx]
    return W_


def build_w_int(wt):
    W_ = np.zeros((3, 120, 96), np.float32)
    for ky in range(3):
        for xa in range(10):
            for xo in range(8):
                kx = xa - xo
                if 0 <= kx <= 2:
                    k = _rloc(xa, 0)
                    W_[ky, k : k + 12, xo * 12 : xo * 12 + 12] = wt[:, :, ky, kx].T
    return W_


def build_w_sums():
    wse = np.zeros((96, 8), np.float32)
    wsep = np.zeros((96, 16), np.float32)
    wbc = np.zeros((8, 16), np.float32)
    for xo in range(8):
        for co in range(CH):
            wse[xo * 12 + co, xo] = 1.0
            wsep[xo * 12 + co, xo * 2 + (co >= 6)] = 1.0
        wbc[xo, xo * 2 : xo * 2 + 2] = 1.0
    return wse, wsep, wbc


def build_w_chroma(cw0):
    wchk = np.zeros((3, 20, 48), np.float32)
    for ky in range(3):
        for xa in range(10):
            for xo in range(8):
                kx = xa - xo
                if 0 <= kx <= 2:
                    for co in range(6):
                        for d in range(2):
                            wchk[ky, _rbloc(xa, d), xo * 6 + co] = cw0[co, d, ky, kx]
    # green_add = [m0, g1, m3, m0, g0, m3]; g0 = m1 - d0, g1 = m2 - d1
    for xo in range(8):
        wchk[1, _rbloc(xo + 1, 1), xo * 6 + 1] += -1.0
        wchk[1, _rbloc(xo + 1, 0), xo * 6 + 4] += -1.0
    wchm = np.zeros((128, 48), np.float32)
    for xo in range(8):
        xa = xo + 1
        wchm[_r0loc(1, 0, xa), xo * 6 + 0] = 1.0
        wchm[_r0loc(1, 0, xa), xo * 6 + 3] = 1.0
        wchm[_r0loc(1, 3, xa), xo * 6 + 2] = 1.0
        wchm[_r0loc(1, 3, xa), xo * 6 + 5] = 1.0
        wchm[_r0loc(1, 2, xa), xo * 6 + 1] = 1.0
        wchm[_r0loc(1, 1, xa), xo * 6 + 4] = 1.0
    return wchk, wchm


def assemble_output(mosaic, cp_dev, g_dev):
    B = mosaic.shape[0]
    cp = cp_dev.reshape(B, 8, 6, H, NW).transpose(0, 2, 3, 4, 1).reshape(B, 6, H, W)
    g = g_dev.reshape(B, 8, 2, H, NW).transpose(0, 2, 3, 4, 1).reshape(B, 2, H, W)
    m = mosaic
    out = np.empty((B, 3, 2 * H, 2 * W), np.float32)
    out[:, 0, 0::2, 0::2] = cp[:, 0]
    out[:, 0, 0::2, 1::2] = m[:, 1]
    out[:, 0, 1::2, 0::2] = cp[:, 1]
    out[:, 0, 1::2, 1::2] = cp[:, 2]
    out[:, 1, 0::2, 0::2] = m[:, 0]
    out[:, 1, 0::2, 1::2] = g[:, 0]
    out[:, 1, 1::2, 0::2] = g[:, 1]
    out[:, 1, 1::2, 1::2] = m[:, 3]
    out[:, 2, 0::2, 0::2] = cp[:, 3]
    out[:, 2, 0::2, 1::2] = cp[:, 4]
    out[:, 2, 1::2, 0::2] = m[:, 2]
    out[:, 2, 1::2, 1::2] = cp[:, 5]
    return out


def _mm_dt(ap):
    return ap.bitcast(F32R) if USE_F32R else ap


# column offsets inside the packed [128, 1576] stationary tensor
_WOFF = {"wf0": 0, "ww0": 96, "wf1": 192, "wf2": 480, "ww1": 768, "ww2": 1056,
         "wse": 1344, "wsep": 1352, "wbc": 1368, "wchk": 1384, "wchm": 1528}
_WCOLS = 1576


def pack_stationaries(st):
    wp = np.zeros((128, _WCOLS), np.float32)
    wp[:, 0:96] = st["wf0"]
    wp[:, 96:192] = st["ww0"]
    for nm in ("wf1", "wf2", "ww1", "ww2"):
        o = _WOFF[nm]
        for ky in range(3):
            wp[0:120, o + 96 * ky : o + 96 * (ky + 1)] = st[nm][ky]
    wp[0:96, 1344:1352] = st["wse"]
    wp[0:96, 1352:1368] = st["wsep"]
    wp[0:8, 1368:1384] = st["wbc"]
    for ky in range(3):
        wp[0:20, 1384 + 48 * ky : 1384 + 48 * (ky + 1)] = st["wchk"][ky]
    wp[:, 1528:1576] = st["wchm"]
    return wp


_W_SHAPES = [
    ("wf0", [128, 96]),
    ("ww0", [128, 96]),
    ("wf1", [120, 3, 96]),
    ("wf2", [120, 3, 96]),
    ("ww1", [120, 3, 96]),
    ("ww2", [120, 3, 96]),
    ("wse", [96, 8]),
    ("wsep", [96, 16]),
    ("wbc", [8, 16]),
    ("wchk", [20, 3, 48]),
    ("wchm", [128, 48]),
]


def build_program():
    from contextlib import ExitStack

    nc = bacc.Bacc(
        "TRN2", target_bir_lowering=False, debug=False, num_devices=N_CORES
    )
    r0 = nc.declare_dram_parameter("r0", [B_PC, 128, H, NW], F32, isOutput=False)
    wpack = nc.declare_dram_parameter("wpack", [128, _WCOLS], F32, isOutput=False)
    out_cp = nc.declare_dram_parameter("out_cp", [B_PC, 48, H, NW], F32, isOutput=True)
    out_g = nc.declare_dram_parameter("out_g", [B_PC, 16, H, NW], F32, isOutput=True)

    Relu = mybir.ActivationFunctionType.Relu
    Exp = mybir.ActivationFunctionType.Exp
    Copy = mybir.ActivationFunctionType.Copy
    NSTEPS = B_PC * NSLAB

    with tile.TileContext(nc) as tc, ExitStack() as ctx:
        const = ctx.enter_context(tc.tile_pool(name="const", bufs=1))
        r0pool = ctx.enter_context(tc.tile_pool(name="r0pool", bufs=6))
        p_rf1 = ctx.enter_context(tc.tile_pool(name="rf1", bufs=4))
        p_rw1 = ctx.enter_context(tc.tile_pool(name="rw1", bufs=4))
        p_rf2 = ctx.enter_context(tc.tile_pool(name="rf2", bufs=4))
        p_rw2 = ctx.enter_context(tc.tile_pool(name="rw2", bufs=4))
        p_grb = ctx.enter_context(tc.tile_pool(name="grb", bufs=4))
        p_d = ctx.enter_context(tc.tile_pool(name="dbuf", bufs=2))
        p_act = ctx.enter_context(tc.tile_pool(name="acts", bufs=3))
        p_stg = ctx.enter_context(tc.tile_pool(name="stg", bufs=3))
        ps_mm = ctx.enter_context(tc.tile_pool(name="psmm", bufs=4, space="PSUM"))
        ps_sm = ctx.enter_context(tc.tile_pool(name="pssm", bufs=2, space="PSUM"))
        ps_cp = ctx.enter_context(tc.tile_pool(name="pscp", bufs=2, space="PSUM"))

        WC = const.tile([128, _WCOLS], F32, tag="wpack_sb", name="wpack_sb")
        nc.sync.dma_start(out=WC[:], in_=wpack[:])
        sb = {
            "wf0": WC[:, 0:96],
            "ww0": WC[:, 96:192],
            "wse": WC[0:96, 1344:1352],
            "wsep": WC[0:96, 1352:1368],
            "wbc": WC[0:8, 1368:1384],
            "wchm": WC[:, 1528:1576],
        }

        def wky(nm, ky):
            o = _WOFF[nm]
            if nm == "wchk":
                return WC[0:20, o + 48 * ky : o + 48 * (ky + 1)]
            return WC[0:120, o + 96 * ky : o + 96 * (ky + 1)]

        r0s, rf1, rw1, rf2, rw2, grb = {}, {}, {}, {}, {}, {}

        def get_rbuf(pool, dct, s):
            if s in dct or not (0 <= s < NSTEPS):
                return dct.get(s)
            t = pool.tile([120, 10, NW], F32)
            dct[s] = t
            sl = s % NSLAB
            if sl == 0:
                nc.vector.memset(t[:, 0:1, :], 0.0)
            if sl == NSLAB - 1:
                nc.vector.memset(t[:, 9:10, :], 0.0)
            nc.vector.memset(t[96:120, :, 0:1], 0.0)
            nc.vector.memset(t[96:120, :, 63:64], 0.0)
            return t

        def get_grb(s):
            if s in grb or not (0 <= s < NSTEPS):
                return grb.get(s)
            t = p_grb.tile([20, 10, NW], F32, name="g")
            grb[s] = t
            sl = s % NSLAB
            if sl == 0:
                nc.vector.memset(t[:, 0:1, :], 0.0)
            if sl == NSLAB - 1:
                nc.vector.memset(t[:, 9:10, :], 0.0)
            nc.vector.memset(t[:, :, 0:1], 0.0)
            nc.vector.memset(t[:, :, 63:64], 0.0)
            return t

        def conv_int(nm, rbuf):
            ps = ps_mm.tile([96, 8, NW], F32, tag="mm96", name="psc")
            for ky in range(3):
                nc.tensor.matmul(
                    ps[:],
                    _mm_dt(wky(nm, ky)),
                    _mm_dt(rbuf[:, ky : ky + 8, :]),
                    start=(ky == 0),
                    stop=(ky == 2),
                )
            return ps

        def evict(ps, dct, s):
            sl = s % NSLAB
            nc.scalar.activation(out=dct[s][0:96, 1:9, :], in_=ps[:], func=Relu)
            if sl < NSLAB - 1:
                nc.scalar.activation(
                    out=dct[s + 1][0:96, 0:1, :], in_=ps[:, 7:8, :], func=Relu
                )
            if sl > 0:
                nc.scalar.activation(
                    out=dct[s - 1][0:96, 9:10, :], in_=ps[:, 0:1, :], func=Relu
                )

        def strips(t):
            nc.sync.dma_start(out=t[96:108, :, 1:NW], in_=t[84:96, :, 0 : NW - 1])
            nc.sync.dma_start(out=t[108:120, :, 0 : NW - 1], in_=t[0:12, :, 1:NW])

        for T in range(NSTEPS + 3):
            s0 = T
            if 0 <= s0 < NSTEPS:
                img, sl = divmod(s0, NSLAB)
                y0 = sl * 8
                rt = r0pool.tile([128, 8, NW], F32, name="rt")
                r0s[s0] = rt
                nc.sync.dma_start(out=rt[:], in_=r0[img, :, y0 : y0 + 8, :])
                get_rbuf(p_rf1, rf1, s0)
                get_rbuf(p_rf1, rf1, s0 + 1)
                get_rbuf(p_rw1, rw1, s0)
                get_rbuf(p_rw1, rw1, s0 + 1)
                psf = ps_mm.tile([96, 8, NW], F32, tag="mm96", name="psf0")
                nc.tensor.matmul(
                    psf[:], _mm_dt(sb["wf0"]), _mm_dt(rt[:]), start=True, stop=True
                )
                evict(psf, rf1, s0)
                psw = ps_mm.tile([96, 8, NW], F32, tag="mm96", name="psw0")
                nc.tensor.matmul(
                    psw[:], _mm_dt(sb["ww0"]), _mm_dt(rt[:]), start=True, stop=True
                )
                evict(psw, rw1, s0)

            s1 = T - 1
            if 0 <= s1 < NSTEPS:
                strips(rf1[s1])
                strips(rw1[s1])
                get_rbuf(p_rf2, rf2, s1)
                get_rbuf(p_rf2, rf2, s1 + 1)
                get_rbuf(p_rw2, rw2, s1)
                get_rbuf(p_rw2, rw2, s1 + 1)
                evict(conv_int("wf1", rf1[s1]), rf2, s1)
                evict(conv_int("ww1", rw1[s1]), rw2, s1)

            s2 = T - 2
            if 0 <= s2 < NSTEPS:
                strips(rf2[s2])
                strips(rw2[s2])
                psf = conv_int("wf2", rf2[s2])
                psw = conv_int("ww2", rw2[s2])
                P = p_act.tile([96, 8, NW], F32, tag="P", name="P")
                nc.scalar.activation(out=P[:], in_=psf[:], func=Relu)
                Et = p_act.tile([96, 8, NW], F32, tag="Et", name="Et")
                nc.scalar.activation(out=Et[:], in_=psw[:], func=Relu)
                E = p_act.tile([96, 8, NW], F32, tag="E", name="E")
                nc.scalar.activation(out=E[:], in_=Et[:], func=Exp)
                EP = p_act.tile([96, 8, NW], F32, tag="EP", name="EP")
                nc.vector.tensor_mul(EP[:], E[:], P[:])
                pse = ps_sm.tile([8, 8, NW], F32, tag="sm", name="pse")
                nc.tensor.matmul(
                    pse[:], _mm_dt(sb["wse"]), _mm_dt(E[:]), start=True, stop=True
                )
                psep = ps_sm.tile([16, 8, NW], F32, tag="sm", name="psep")
                nc.tensor.matmul(
                    psep[:], _mm_dt(sb["wsep"]), _mm_dt(EP[:]), start=True, stop=True
                )
                rcp = p_act.tile([8, 8, NW], F32, tag="rcp", name="rcp")
                nc.vector.reciprocal(out=rcp[:], in_=pse[:])
                psbc = ps_sm.tile([16, 8, NW], F32, tag="sm", name="psbc")
                nc.tensor.matmul(
                    psbc[:], _mm_dt(sb["wbc"]), _mm_dt(rcp[:]), start=True, stop=True
                )
                bcs = p_act.tile([16, 8, NW], F32, tag="bcs", name="bcs")
                nc.scalar.activation(out=bcs[:], in_=psbc[:], func=Copy)
                get_grb(s2)
                get_grb(s2 + 1)
                g = grb[s2]
                nc.vector.tensor_mul(g[0:16, 1:9, :], psep[:], bcs[:])
                sl = s2 % NSLAB
                if sl < NSLAB - 1:
                    nc.vector.tensor_copy(
                        out=grb[s2 + 1][0:16, 0:1, :], in_=g[0:16, 8:9, :]
                    )
                if sl > 0:
                    nc.vector.tensor_copy(
                        out=grb[s2 - 1][0:16, 9:10, :], in_=g[0:16, 1:2, :]
                    )

            s3 = T - 3
            if 0 <= s3 < NSTEPS:
                img, sl = divmod(s3, NSLAB)
                y0 = sl * 8
                g = grb[s3]
                nc.sync.dma_start(out=g[16:18, :, 1:NW], in_=g[14:16, :, 0 : NW - 1])
                nc.sync.dma_start(out=g[18:20, :, 0 : NW - 1], in_=g[0:2, :, 1:NW])
                rt = r0s[s3]
                d = p_d.tile([20, 10, NW], F32, name="d")
                nc.vector.tensor_copy(out=d[:, 1:9, :], in_=rt[64:84, :, :])
                if sl > 0:
                    nc.vector.tensor_copy(
                        out=d[:, 0:1, :], in_=r0s[s3 - 1][64:84, 7:8, :]
                    )
                else:
                    nc.vector.memset(d[:, 0:1, :], 0.0)
                if sl < NSLAB - 1:
                    nc.vector.tensor_copy(
                        out=d[:, 9:10, :], in_=r0s[s3 + 1][64:84, 0:1, :]
                    )
                else:
                    nc.vector.memset(d[:, 9:10, :], 0.0)
                nc.vector.tensor_sub(d[:], d[:], g[:])
                pc = ps_cp.tile([48, 8, NW], F32, tag="cp", name="pc")
                for ky in range(3):
                    nc.tensor.matmul(
                        pc[:],
                        _mm_dt(wky("wchk", ky)),
                        _mm_dt(d[:, ky : ky + 8, :]),
                        start=(ky == 0),
                        stop=False,
                    )
                nc.tensor.matmul(
                    pc[:], _mm_dt(sb["wchm"]), _mm_dt(rt[:]), start=False, stop=True
                )
                stg = p_stg.tile([48, 8, NW], F32, name="stg")
                nc.scalar.activation(out=stg[:], in_=pc[:], func=Copy)
                nc.sync.dma_start(out=out_cp[img, :, y0 : y0 + 8, :], in_=stg[:])
                nc.sync.dma_start(out=out_g[img, :, y0 : y0 + 8, :], in_=g[0:16, 1:9, :])
                for dct in (r0s, rf1, rw1, rf2, rw2, grb):
                    dct.pop(s3 - 2, None)

    nc.compile()
    return nc


_CACHE = {}


def kernel(mosaic, fw0, fw1, fw2, ww0, ww1, ww2, cw0, _trace=False):
    mosaic = np.asarray(mosaic, np.float32)
    r0_all = build_r0(mosaic)

    stat = {
        "wf0": build_w_l0(np.asarray(fw0, np.float32)),
        "ww0": build_w_l0(np.asarray(ww0, np.float32)),
        "wf1": build_w_int(np.asarray(fw1, np.float32)),
        "wf2": build_w_int(np.asarray(fw2, np.float32)),
        "ww1": build_w_int(np.asarray(ww1, np.float32)),
        "ww2": build_w_int(np.asarray(ww2, np.float32)),
    }
    stat["wse"], stat["wsep"], stat["wbc"] = build_w_sums()
    stat["wchk"], stat["wchm"] = build_w_chroma(np.asarray(cw0, np.float32))
    wpack = pack_stationaries(stat)

    if "nc" not in _CACHE:
        _CACHE["nc"] = build_program()
    nc = _CACHE["nc"]

    in_maps = []
    for c in range(N_CORES):
        in_maps.append(
            {"r0": np.ascontiguousarray(r0_all[c * B_PC : (c + 1) * B_PC]),
             "wpack": wpack}
        )

    res = run_bass_kernel_spmd(nc, in_maps, list(range(N_CORES)), trace=_trace)
    outs = []
    for c in range(N_CORES):
        outs.append(
            assemble_output(
                mosaic[c * B_PC : (c + 1) * B_PC],
                res.results[c]["out_cp"],
                res.results[c]["out_g"],
            )
        )
    full = np.concatenate(outs, axis=0)
    if _trace:
        return full, res
    return full



# revision 2
# speedup vs baseline: 2.9007x; 2.9007x over previous
"""Trainium2 Bass kernel for BasicQuadRGBModel (quad-Bayer demosaic CNN).

v3 design (bf16 matmuls at 1 cyc/row, group-of-4-slab tiles):
  - data parallel over 8 cores, 2 images each; per-image 16 groups of
    32 rows (4 slabs x 8 rows); pipeline stages A..D at group granularity.
  - im2col layout as v2: conv = accumulating matmuls with K=120
    (10 xa x 12 ch band packing), M=96 (8 xo x 12 ch), N=512.
  - all matmul operands bf16 (PSUM accum fp32): 4x faster than fp32.
  - softmax: 16-wide sum matmuls (wse16/wsep) + DVE reciprocal; the old
    wbc broadcast matmul and bcs copy are folded away.
  - E = max(exp(x), 1) replaces relu+exp (one ACT op + cheap DVE max).
  - chroma: one matmul per slab over a [104p] combined buffer (3 row-
    shifted copies of d = rb - g, plus the ky1 rows of r0) built by DMA.
  - engine split: ACT = conv evicts + exp + chroma copy; DVE = P/E max,
    EP, reciprocal, g mul, d sub; DMA = strips/halos/combine/IO.
  - host does layer-0 im2col (bf16) and the final 2x2 pixel-shuffle.
"""

import sys

sys.path.insert(0, "/opt/trn_rl_repo")

import ml_dtypes
import numpy as np

import concourse.bass as bass  # noqa: F401
import concourse.mybir as mybir
import concourse.tile as tile
from concourse import bacc
from concourse.bass_utils import run_bass_kernel_spmd

N_CORES = 8
B_PC = 2
H = W = 512
NW = 64
CH = 12
GS = 4
GROWS = GS * 8
NG_IMG = H // GROWS
NGROUP = B_PC * NG_IMG
F32 = mybir.dt.float32
BF16 = mybir.dt.bfloat16
BF16NP = ml_dtypes.bfloat16


def _rbloc(xa, c):
    if xa == 0:
        return 16 + c
    if xa == 9:
        return 18 + c
    return (xa - 1) * 2 + c


def _rloc(xa, ci):
    if xa == 0:
        return 96 + ci
    if xa == 9:
        return 108 + ci
    return (xa - 1) * 12 + ci


def _r0loc(ky, ci, xa):
    if ky == 0:
        if ci == 0:
            return xa
        if ci == 3:
            return 10 + xa
        return 20 + _rbloc(xa, ci - 1)
    if ky == 1:
        if ci == 0:
            return 40 + xa
        if ci == 3:
            return 50 + xa
        return 64 + _rbloc(xa, ci - 1)
    if ci == 0:
        return 84 + xa
    if ci == 3:
        return 94 + xa
    return 104 + _rbloc(xa, ci - 1)


def build_r0(mosaic):
    B = mosaic.shape[0]
    mp = np.zeros((B, 4, H + 2, W + 2), BF16NP)
    mp[:, :, 1 : H + 1, 1 : W + 1] = mosaic.astype(BF16NP)
    r0 = np.zeros((B, 128, H, NW), BF16NP)
    for ky in range(3):
        for ci in range(4):
            for xa in range(10):
                r0[:, _r0loc(ky, ci, xa)] = mp[:, ci, ky : ky + H, xa : xa + 8 * NW : 8]
    return r0


def build_w_l0(wt):
    W_ = np.zeros((128, 96), np.float32)
    for ky in range(3):
        for ci in range(4):
            for xa in range(10):
                for xo in range(8):
                    kx = xa - xo
                    if 0 <= kx <= 2:
                        for co in range(CH):
                            W_[_r0loc(ky, ci, xa), xo * 12 + co] = wt[co, ci, ky, kx]
    return W_


def build_w_int(wt):
    W_ = np.zeros((3, 120, 96), np.float32)
    for ky in range(3):
        for xa in range(10):
            for xo in range(8):
                kx = xa - xo
                if 0 <= kx <= 2:
                    k = _rloc(xa, 0)
                    W_[ky, k : k + 12, xo * 12 : xo * 12 + 12] = wt[:, :, ky, kx].T
    return W_


def build_w_sums16():
    wse16 = np.zeros((96, 16), np.float32)
    wsep = np.zeros((96, 16), np.float32)
    for xo in range(8):
        for co in range(CH):
            wse16[xo * 12 + co, 2 * xo] = 1.0
            wse16[xo * 12 + co, 2 * xo + 1] = 1.0
            wsep[xo * 12 + co, xo * 2 + (co >= 6)] = 1.0
    return wse16, wsep


def build_w_chroma_comb(cw0):
    wchk = np.zeros((3, 20, 48), np.float32)
    for ky in range(3):
        for xa in range(10):
            for xo in range(8):
                kx = xa - xo
                if 0 <= kx <= 2:
                    for co in range(6):
                        for d in range(2):
                            wchk[ky, _rbloc(xa, d), xo * 6 + co] = cw0[co, d, ky, kx]
    # green_add = [m0, g1, m3, m0, g0, m3]; g0 = m1 - d0, g1 = m2 - d1
    for xo in range(8):
        wchk[1, _rbloc(xo + 1, 1), xo * 6 + 1] += -1.0
        wchk[1, _rbloc(xo + 1, 0), xo * 6 + 4] += -1.0
    wchm = np.zeros((128, 48), np.float32)
    for xo in range(8):
        xa = xo + 1
        wchm[_r0loc(1, 0, xa), xo * 6 + 0] = 1.0
        wchm[_r0loc(1, 0, xa), xo * 6 + 3] = 1.0
        wchm[_r0loc(1, 3, xa), xo * 6 + 2] = 1.0
        wchm[_r0loc(1, 3, xa), xo * 6 + 5] = 1.0
        wchm[_r0loc(1, 2, xa), xo * 6 + 1] = 1.0
        wchm[_r0loc(1, 1, xa), xo * 6 + 4] = 1.0
    Wc = np.zeros((104, 48), np.float32)
    for k in range(3):
        Wc[20 * k : 20 * k + 20] = wchk[k]
    Wc[60:104] = wchm[40:84]
    return Wc


def assemble_output(mosaic, cp_dev, g_dev):
    B = mosaic.shape[0]
    cp = (
        cp_dev.astype(np.float32)
        .reshape(B, 8, 6, H, NW)
        .transpose(0, 2, 3, 4, 1)
        .reshape(B, 6, H, W)
    )
    g = g_dev.reshape(B, 8, 2, H, NW).transpose(0, 2, 3, 4, 1).reshape(B, 2, H, W)
    m = mosaic
    out = np.empty((B, 3, 2 * H, 2 * W), np.float32)
    out[:, 0, 0::2, 0::2] = cp[:, 0]
    out[:, 0, 0::2, 1::2] = m[:, 1]
    out[:, 0, 1::2, 0::2] = cp[:, 1]
    out[:, 0, 1::2, 1::2] = cp[:, 2]
    out[:, 1, 0::2, 0::2] = m[:, 0]
    out[:, 1, 0::2, 1::2] = g[:, 0]
    out[:, 1, 1::2, 0::2] = g[:, 1]
    out[:, 1, 1::2, 1::2] = m[:, 3]
    out[:, 2, 0::2, 0::2] = cp[:, 3]
    out[:, 2, 0::2, 1::2] = cp[:, 4]
    out[:, 2, 1::2, 0::2] = m[:, 2]
    out[:, 2, 1::2, 1::2] = cp[:, 5]
    return out


# column offsets inside the packed [128, 1424] stationary tensor
_WOFF = {"wf0": 0, "ww0": 96, "wf1": 192, "wf2": 480, "ww1": 768, "ww2": 1056,
         "wse16": 1344, "wsep": 1360, "wcomb": 1376}
_WCOLS = 1424


def pack_stationaries(st):
    wp = np.zeros((128, _WCOLS), np.float32)
    wp[:, 0:96] = st["wf0"]
    wp[:, 96:192] = st["ww0"]
    for nm in ("wf1", "wf2", "ww1", "ww2"):
        o = _WOFF[nm]
        for ky in range(3):
            wp[0:120, o + 96 * ky : o + 96 * (ky + 1)] = st[nm][ky]
    wp[0:96, 1344:1360] = st["wse16"]
    wp[0:96, 1360:1376] = st["wsep"]
    wp[0:104, 1376:1424] = st["wcomb"]
    return wp


def build_program():
    from contextlib import ExitStack

    nc = bacc.Bacc(
        "TRN2", target_bir_lowering=False, debug=False, num_devices=N_CORES
    )
    r0 = nc.declare_dram_parameter("r0", [B_PC, 128, H, NW], BF16, isOutput=False)
    wpack = nc.declare_dram_parameter("wpack", [128, _WCOLS], BF16, isOutput=False)
    out_cp = nc.declare_dram_parameter("out_cp", [B_PC, 48, H, NW], BF16, isOutput=True)
    out_g = nc.declare_dram_parameter("out_g", [B_PC, 16, H, NW], F32, isOutput=True)

    Relu = mybir.ActivationFunctionType.Relu
    Exp = mybir.ActivationFunctionType.Exp
    Copy = mybir.ActivationFunctionType.Copy

    with tile.TileContext(nc) as tc, ExitStack() as ctx:
        const = ctx.enter_context(tc.tile_pool(name="const", bufs=1))
        r0pool = ctx.enter_context(tc.tile_pool(name="r0pool", bufs=6))
        p_rf1 = ctx.enter_context(tc.tile_pool(name="rf1", bufs=3))
        p_rw1 = ctx.enter_context(tc.tile_pool(name="rw1", bufs=3))
        p_rf2 = ctx.enter_context(tc.tile_pool(name="rf2", bufs=3))
        p_rw2 = ctx.enter_context(tc.tile_pool(name="rw2", bufs=3))
        p_grb = ctx.enter_context(tc.tile_pool(name="grb", bufs=3))
        p_d = ctx.enter_context(tc.tile_pool(name="dbuf", bufs=2))
        p_comb = ctx.enter_context(tc.tile_pool(name="comb", bufs=2))
        p_act = ctx.enter_context(tc.tile_pool(name="acts", bufs=3))
        p_rcp = ctx.enter_context(tc.tile_pool(name="rcp", bufs=2))
        p_stg = ctx.enter_context(tc.tile_pool(name="stg", bufs=2))
        ps_mm = ctx.enter_context(tc.tile_pool(name="psmm", bufs=4, space="PSUM"))
        ps_sm = ctx.enter_context(tc.tile_pool(name="pssm", bufs=2, space="PSUM"))
        ps_cp = ctx.enter_context(tc.tile_pool(name="pscp", bufs=2, space="PSUM"))

        WC = const.tile([128, _WCOLS], BF16, tag="wpack_sb", name="wpack_sb")
        nc.sync.dma_start(out=WC[:], in_=wpack[:])
        sb = {
            "wf0": WC[:, 0:96],
            "ww0": WC[:, 96:192],
            "wse16": WC[0:96, 1344:1360],
            "wsep": WC[0:96, 1360:1376],
            "wcomb": WC[0:104, 1376:1424],
        }

        def wky(nm, ky):
            o = _WOFF[nm]
            return WC[0:120, o + 96 * ky : o + 96 * (ky + 1)]

        r0s, rf1, rw1, rf2, rw2, grb = {}, {}, {}, {}, {}, {}

        def get_rbuf(pool, dct, g):
            if g in dct or not (0 <= g < NGROUP):
                return dct.get(g)
            t = pool.tile([120, GROWS + 2, NW], BF16)
            dct[g] = t
            gi = g % NG_IMG
            if gi == 0:
                nc.vector.memset(t[0:96, 0:1, :], 0.0)
            if gi == NG_IMG - 1:
                nc.vector.memset(t[0:96, GROWS + 1 : GROWS + 2, :], 0.0)
            nc.vector.memset(t[96:120, :, 0:1], 0.0)
            nc.vector.memset(t[96:120, :, NW - 1 : NW], 0.0)
            return t

        def get_grb(g):
            if g in grb or not (0 <= g < NGROUP):
                return grb.get(g)
            t = p_grb.tile([20, GROWS + 2, NW], F32, name="g")
            grb[g] = t
            gi = g % NG_IMG
            if gi == 0:
                nc.vector.memset(t[:, 0:1, :], 0.0)
            if gi == NG_IMG - 1:
                nc.vector.memset(t[:, GROWS + 1 : GROWS + 2, :], 0.0)
            nc.vector.memset(t[:, :, 0:1], 0.0)
            nc.vector.memset(t[:, :, NW - 1 : NW], 0.0)
            return t

        def evict_g(ps, dct, g, gi, s):
            nc.scalar.activation(
                out=dct[g][0:96, 8 * s + 1 : 8 * s + 9, :], in_=ps[:], func=Relu
            )
            if s == 0 and gi > 0:
                nc.scalar.activation(
                    out=dct[g - 1][0:96, GROWS + 1 : GROWS + 2, :],
                    in_=ps[:, 0:1, :],
                    func=Relu,
                )
            if s == GS - 1 and gi < NG_IMG - 1:
                nc.scalar.activation(
                    out=dct[g + 1][0:96, 0:1, :], in_=ps[:, 7:8, :], func=Relu
                )

        def strips(t):
            nc.sync.dma_start(out=t[96:108, :, 1:NW], in_=t[84:96, :, 0 : NW - 1])
            nc.sync.dma_start(out=t[108:120, :, 0 : NW - 1], in_=t[0:12, :, 1:NW])

        for T in range(NGROUP + 3):
            a = T
            if 0 <= a < NGROUP:
                img, gi = divmod(a, NG_IMG)
                rt = r0pool.tile([128, GROWS, NW], BF16, name="rt")
                r0s[a] = rt
                nc.sync.dma_start(
                    out=rt[:], in_=r0[img, :, gi * GROWS : (gi + 1) * GROWS, :]
                )
                get_rbuf(p_rf1, rf1, a)
                get_rbuf(p_rf1, rf1, a + 1)
                get_rbuf(p_rw1, rw1, a)
                get_rbuf(p_rw1, rw1, a + 1)
                for s in range(GS):
                    for nm, dct in (("wf0", rf1), ("ww0", rw1)):
                        ps = ps_mm.tile([96, 8, NW], F32, tag="mm96", name="ps0")
                        nc.tensor.matmul(
                            ps[:],
                            sb[nm],
                            rt[:, 8 * s : 8 * s + 8, :],
                            start=True,
                            stop=True,
                        )
                        evict_g(ps, dct, a, gi, s)

            b = T - 1
            if 0 <= b < NGROUP:
                img, gi = divmod(b, NG_IMG)
                strips(rf1[b])
                strips(rw1[b])
                get_rbuf(p_rf2, rf2, b)
                get_rbuf(p_rf2, rf2, b + 1)
                get_rbuf(p_rw2, rw2, b)
                get_rbuf(p_rw2, rw2, b + 1)
                for s in range(GS):
                    for nm, src, dct in (("wf1", rf1[b], rf2), ("ww1", rw1[b], rw2)):
                        ps = ps_mm.tile([96, 8, NW], F32, tag="mm96", name="ps1")
                        for ky in range(3):
                            nc.tensor.matmul(
                                ps[:],
                                wky(nm, ky),
                                src[:, 8 * s + ky : 8 * s + ky + 8, :],
                                start=(ky == 0),
                                stop=(ky == 2),
                            )
                        evict_g(ps, dct, b, gi, s)

            c = T - 2
            if 0 <= c < NGROUP:
                img, gi = divmod(c, NG_IMG)
                strips(rf2[c])
                strips(rw2[c])
                gt = get_grb(c)
                get_grb(c + 1)
                for s in range(GS):
                    psf = ps_mm.tile([96, 8, NW], F32, tag="mm96", name="psf2")
                    for ky in range(3):
                        nc.tensor.matmul(
                            psf[:],
                            wky("wf2", ky),
                            rf2[c][:, 8 * s + ky : 8 * s + ky + 8, :],
                            start=(ky == 0),
                            stop=(ky == 2),
                        )
                    psw = ps_mm.tile([96, 8, NW], F32, tag="mm96", name="psw2")
                    for ky in range(3):
                        nc.tensor.matmul(
                            psw[:],
                            wky("ww2", ky),
                            rw2[c][:, 8 * s + ky : 8 * s + ky + 8, :],
                            start=(ky == 0),
                            stop=(ky == 2),
                        )
                    P = p_act.tile([96, 8, NW], BF16, tag="P", name="P")
                    nc.vector.tensor_scalar_max(P[:], psf[:], 0.0)
                    E0 = p_act.tile([96, 8, NW], BF16, tag="E0", name="E0")
                    nc.scalar.activation(out=E0[:], in_=psw[:], func=Exp)
                    E = p_act.tile([96, 8, NW], BF16, tag="E", name="E")
                    nc.vector.tensor_scalar_max(E[:], E0[:], 1.0)
                    EP = p_act.tile([96, 8, NW], BF16, tag="EP", name="EP")
                    nc.vector.tensor_mul(EP[:], E[:], P[:])
                    pse = ps_sm.tile([16, 8, NW], F32, tag="sm", name="pse")
                    nc.tensor.matmul(pse[:], sb["wse16"], E[:], start=True, stop=True)
                    psep = ps_sm.tile([16, 8, NW], F32, tag="sm", name="psep")
                    nc.tensor.matmul(psep[:], sb["wsep"], EP[:], start=True, stop=True)
                    rcp = p_rcp.tile([16, 8, NW], F32, tag="rcp", name="rcp")
                    nc.vector.reciprocal(out=rcp[:], in_=pse[:])
                    nc.vector.tensor_mul(
                        gt[0:16, 8 * s + 1 : 8 * s + 9, :], psep[:], rcp[:]
                    )
                    if s == 0 and gi > 0:
                        nc.vector.tensor_mul(
                            grb[c - 1][0:16, GROWS + 1 : GROWS + 2, :],
                            psep[:, 0:1, :],
                            rcp[:, 0:1, :],
                        )
                    if s == GS - 1 and gi < NG_IMG - 1:
                        nc.vector.tensor_mul(
                            grb[c + 1][0:16, 0:1, :],
                            psep[:, 7:8, :],
                            rcp[:, 7:8, :],
                        )

            g = T - 3
            if 0 <= g < NGROUP:
                img, gi = divmod(g, NG_IMG)
                gt = grb[g]
                nc.sync.dma_start(out=gt[16:18, :, 1:NW], in_=gt[14:16, :, 0 : NW - 1])
                nc.sync.dma_start(out=gt[18:20, :, 0 : NW - 1], in_=gt[0:2, :, 1:NW])
                dt = p_d.tile([20, GROWS + 2, NW], BF16, name="d")
                nc.sync.dma_start(out=dt[:, 1 : GROWS + 1, :], in_=r0s[g][64:84, :, :])
                if gi > 0:
                    nc.sync.dma_start(
                        out=dt[:, 0:1, :],
                        in_=r0s[g - 1][64:84, GROWS - 1 : GROWS, :],
                    )
                else:
                    nc.vector.memset(dt[:, 0:1, :], 0.0)
                if gi < NG_IMG - 1:
                    nc.sync.dma_start(
                        out=dt[:, GROWS + 1 : GROWS + 2, :],
                        in_=r0s[g + 1][64:84, 0:1, :],
                    )
                else:
                    nc.vector.memset(dt[:, GROWS + 1 : GROWS + 2, :], 0.0)
                nc.vector.tensor_sub(dt[:], dt[:], gt[:])
                cb = p_comb.tile([104, GROWS, NW], BF16, name="cb")
                for k in range(3):
                    nc.sync.dma_start(
                        out=cb[20 * k : 20 * (k + 1), :, :], in_=dt[:, k : k + GROWS, :]
                    )
                nc.sync.dma_start(out=cb[60:104, :, :], in_=r0s[g][40:84, :, :])
                stgt = p_stg.tile([48, GROWS, NW], BF16, name="stg")
                for s in range(GS):
                    pc = ps_cp.tile([48, 8, NW], F32, tag="cp", name="pc")
                    nc.tensor.matmul(
                        pc[:],
                        sb["wcomb"],
                        cb[:, 8 * s : 8 * s + 8, :],
                        start=True,
                        stop=True,
                    )
                    nc.scalar.activation(
                        out=stgt[:, 8 * s : 8 * s + 8, :], in_=pc[:], func=Copy
                    )
                y0 = gi * GROWS
                nc.sync.dma_start(out=out_cp[img, :, y0 : y0 + GROWS, :], in_=stgt[:])
                nc.sync.dma_start(
                    out=out_g[img, :, y0 : y0 + GROWS, :],
                    in_=gt[0:16, 1 : GROWS + 1, :],
                )
                r0s.pop(g - 1, None)
                rf1.pop(T - 1 - 1, None)
                rw1.pop(T - 1 - 1, None)
                rf2.pop(T - 2 - 1, None)
                rw2.pop(T - 2 - 1, None)
                grb.pop(g, None)

    nc.compile()
    return nc


_CACHE = {}


def kernel(mosaic, fw0, fw1, fw2, ww0, ww1, ww2, cw0, _trace=False):
    mosaic = np.asarray(mosaic, np.float32)
    r0_all = build_r0(mosaic)

    stat = {
        "wf0": build_w_l0(np.asarray(fw0, np.float32)),
        "ww0": build_w_l0(np.asarray(ww0, np.float32)),
        "wf1": build_w_int(np.asarray(fw1, np.float32)),
        "wf2": build_w_int(np.asarray(fw2, np.float32)),
        "ww1": build_w_int(np.asarray(ww1, np.float32)),
        "ww2": build_w_int(np.asarray(ww2, np.float32)),
    }
    stat["wse16"], stat["wsep"] = build_w_sums16()
    stat["wcomb"] = build_w_chroma_comb(np.asarray(cw0, np.float32))
    wpack = pack_stationaries(stat).astype(BF16NP)

    if "nc" not in _CACHE:
        _CACHE["nc"] = build_program()
    nc = _CACHE["nc"]

    in_maps = []
    for c in range(N_CORES):
        in_maps.append(
            {"r0": np.ascontiguousarray(r0_all[c * B_PC : (c + 1) * B_PC]),
             "wpack": wpack}
        )

    res = run_bass_kernel_spmd(nc, in_maps, list(range(N_CORES)), trace=_trace)
    outs = []
    for c in range(N_CORES):
        outs.append(
            assemble_output(
                mosaic[c * B_PC : (c + 1) * B_PC],
                res.results[c]["out_cp"],
                res.results[c]["out_g"],
            )
        )
    full = np.concatenate(outs, axis=0)
    if _trace:
        return full, res
    return full
